# revision 33
# baseline (speedup 1.0000x reference)
"""Trainium2 Bass kernel for channel-attention + 2 residual conv blocks.

Data-parallel over batch (8 cores, 1 batch element each). Two SPMD launches:
  A) raw channel Gram G = [x;y]^T-pixel-contraction ([128,128]) via 512
     accumulating matmuls on pixel-major data (host pre-transposes); no
     per-tile PSUM->SBUF copies at all.
  B) fused attention-apply + 4 3x3 convs. Convs run in fp8-e4m3 DoubleRow
     matmuls (2 k-tiles replace the row-shifted duplicate half entirely),
     packed 4 output rows per matmul group (M=128=2rows x 64ch,
     N=512=2 slot-pairs x 256 cols). Residual paths stay exact via bf16
     identity/Wav injection matmuls into PSUM; stage writes balance across
     ACT/DVE (GPSIMD cannot read PSUM) and the four convs are
     software-pipelined in 2-wave-lagged groups per 32-row block. Out rows
     stage parity-paired in SBUF and leave in 16-row DMAs. Host does only
     the O(64^2) softmax/fold algebra between launches.
"""
import sys, os
for p in ('/opt/trn_rl_repo', os.path.expanduser('~/.axon_site/_ro/trn_rl_repo')):
    if os.path.isdir(p) and p not in sys.path:
        sys.path.insert(0, p)

import numpy as np
import ml_dtypes
import concourse.bass as bass
import concourse.bacc as bacc
import concourse.tile as tile
from concourse import mybir
from concourse.bass_utils import run_bass_kernel_spmd

dt = mybir.dt
F32, BF16, FP8 = dt.float32, dt.bfloat16, dt.float8e4
BF = ml_dtypes.bfloat16
E4 = ml_dtypes.float8_e4m3
AF = mybir.ActivationFunctionType
OP = mybir.AluOpType
DR = mybir.MatmulPerfMode.DoubleRow

D = 64
HW = 65536
H = W_IMG = 256
NCORES = 8
R = 32  # output rows per block in pass B


def _build_nc_a():
    nc = bacc.Bacc("TRN2", target_bir_lowering=False, debug=False)
    za = nc.dram_tensor("za", [32, 128, 2048], BF16, kind="ExternalInput").ap()
    gout = nc.dram_tensor("gout", [128, 128], F32, kind="ExternalOutput").ap()
    with tile.TileContext(nc) as tc:
        with tc.tile_pool(name="io", bufs=3) as io, \
             tc.tile_pool(name="work", bufs=1) as work, \
             tc.tile_pool(name="gps", bufs=1, space="PSUM") as gps:
            gp = gps.tile([128, 128], F32)
            for t in range(32):
                zt = io.tile([128, 2048], BF16, tag="zt")
                nc.sync.dma_start(out=zt, in_=za[t])
                for j in range(16):
                    s = zt[:, j * 128:(j + 1) * 128]
                    nc.tensor.matmul(gp, s, s,
                                     start=(t == 0 and j == 0),
                                     stop=(t == 31 and j == 15),
                                     skip_group_check=True)
            gs = work.tile([128, 128], F32)
            nc.vector.tensor_copy(out=gs, in_=gp)
            nc.sync.dma_start(out=gout, in_=gs)
    nc.compile()
    return nc


def _emit_groups(lo, hi):
    """4-row groups (+2-row remainder; odd counts overlap by one row)."""
    out, g, n = [], lo, hi - lo
    while n >= 4:
        out.append((g, 4)); g += 4; n -= 4
    if n == 3:
        out.append((hi - 4, 4))
    elif n == 2:
        out.append((g, 2))
    elif n == 1:
        out.append((hi - 2, 2))
    return out


def _build_nc_b():
    nc = bacc.Bacc("TRN2", target_bir_lowering=False, debug=False)
    yb = nc.dram_tensor("yb", [D, H, W_IMG], BF16, kind="ExternalInput").ap()
    wavt = nc.dram_tensor("wavt", [D, D], BF16, kind="ExternalInput").ap()
    ii_d = nc.dram_tensor("ii", [2 * D, D], BF16, kind="ExternalInput").ap()
    # fp8 DoubleRow weights: per conv, per kx, (a|b) variant [64, 2, 128]
    wdr_d = {}
    for c in range(1, 5):
        for kx in range(3):
            for v in 'ab':
                nm = f"w{c}{kx}{v}"
                wdr_d[nm] = nc.dram_tensor(nm, [D, 2, 2 * D], FP8,
                                           kind="ExternalInput").ap()
    bias_d = {nm: nc.dram_tensor(nm, [D, 1], F32, kind="ExternalInput").ap()
              for nm in ('bt0', 'bc1', 'bo3b', 'bn4', 'bc3')}
    out_d = nc.dram_tensor("out", [D, H, W_IMG], F32, kind="ExternalOutput").ap()

    with tile.TileContext(nc) as tc:
        with tc.tile_pool(name="consts", bufs=1) as consts, \
             tc.tile_pool(name="stg", bufs=1) as stg, \
             tc.tile_pool(name="stgr", bufs=2) as stgr, \
             tc.tile_pool(name="oyp", bufs=2) as oyp, \
             tc.tile_pool(name="outs", bufs=2) as outs, \
             tc.tile_pool(name="ps", bufs=6, space="PSUM") as ps, \
             tc.tile_pool(name="ps2", bufs=2, space="PSUM") as ps2:
            # block-0 y load first: everything on the critical path needs it,
            # and the ~30 const DMAs would otherwise delay it on the SP queue
            oy0 = oyp.tile([2 * D, 40, W_IMG], BF16, tag="oy")
            nc.sync.dma_start(out=oy0[0:D, 4:40, :], in_=yb[:, 0:36, :])

            # const loads issue from the (idle) Pool/ACT DMA queues so they
            # don't serialize behind each other or the y loads on SP
            wavt_t = consts.tile([D, D], BF16)
            ii_t = consts.tile([2 * D, D], BF16)
            nc.scalar.dma_start(out=wavt_t, in_=wavt)
            nc.scalar.dma_start(out=ii_t, in_=ii_d)
            wdr = {}
            for nm, d in wdr_d.items():
                t = consts.tile([D, 2, 2 * D], FP8, tag=nm)
                nc.gpsimd.dma_start(out=t, in_=d)
                wdr[nm] = t
            bias = {}
            for nm, d in bias_d.items():
                t = consts.tile([D, 1], F32, tag=nm)
                nc.scalar.dma_start(out=t, in_=d)
                bias[nm] = t

            # t0 persists across blocks (full image); col 0/257 stay zero
            t0 = stg.tile([D, 258, 258], FP8)   # slot = row + 1
            nc.vector.memset(t0[:, :, 0:1], 0.0)
            nc.vector.memset(t0[:, :, 257:258], 0.0)
            nc.vector.memset(t0[:, 0:1, :], 0.0)     # virtual row -1
            nc.vector.memset(t0[:, 257:258, :], 0.0)  # virtual row 256

            # stage-write engine balance: psum -> stage with bias (+relu).
            # GPSIMD cannot read PSUM, so these go to ACT/DVE, weighted by
            # modeled per-op cost (ACT 0.833ns/el +143, DVE 1.042ns/el +125).
            acc = [0.0, 0.0]

            def wr(out_ap, in_ap, b, relu):
                n = out_ap.free_size()
                ca, cd = n * 0.833 + 143.0, n * 1.042 + 125.0
                if acc[0] + ca <= acc[1] + cd:
                    acc[0] += ca
                    nc.scalar.activation(out=out_ap, in_=in_ap,
                                         func=(AF.Relu if relu else AF.Identity),
                                         bias=b, scale=1.0)
                elif relu:
                    acc[1] += cd
                    nc.vector.tensor_scalar(out=out_ap, in0=in_ap, scalar1=b,
                                            scalar2=0.0, op0=OP.add, op1=OP.max)
                else:
                    acc[1] += cd
                    nc.vector.tensor_scalar_add(out=out_ap, in0=in_ap, scalar1=b)

            # --- block loop -------------------------------------------------
            t0_done = 0  # t0 rows produced so far
            for blk in range(8):
                r0, r1 = blk * R, (blk + 1) * R
                oy = oy0 if blk == 0 else oyp.tile([2 * D, 40, W_IMG], BF16,
                                                   tag="oy")
                # per-block stages, double-buffered for cross-block overlap
                c1 = stgr.tile([D, 40, 258], FP8, tag="c1")
                o3f = stgr.tile([D, 38, 258], FP8, tag="o3f")
                c3 = stgr.tile([D, 36, 258], FP8, tag="c3")
                if blk < 2:  # each rotating buffer's gap cols, zeroed once
                    for t in (c1, o3f, c3):
                        nc.vector.memset(t[:, :, 0:1], 0.0)
                        nc.vector.memset(t[:, :, 257:258], 0.0)

                def oslot(row):
                    return row - (r0 - 4)

                if blk > 0:
                    ylo, yhi = r0 - 4, min(r1 + 4, 256)
                    nc.sync.dma_start(out=oy[0:D, oslot(ylo):oslot(yhi), :],
                                      in_=yb[:, ylo:yhi, :])

                # stage ranges (produced rows)
                c1lo, c1hi = max(r0 - 3, 0), min(r1 + 3, 256)
                o3lo, o3hi = max(r0 - 2, 0), min(r1 + 2, 256)
                c3lo, c3hi = max(r0 - 1, 0), min(r1 + 1, 256)

                def s_c1(row): return row - c1lo + 1
                def s_o3(row): return row - o3lo + 1
                def s_c3(row): return row - c3lo + 1
                def s_t0(row): return row + 1

                # virtual zero rows at image edges (persistent tiles: emit
                # only when the slot is actually consumed as a virtual row)
                if blk == 0:
                    for t in (c1, o3f, c3):
                        nc.vector.memset(t[:, 0:1, :], 0.0)
                if blk == 7:
                    nc.vector.memset(c1[:, s_c1(256):s_c1(256) + 1, :], 0.0)
                    nc.vector.memset(o3f[:, s_o3(256):s_o3(256) + 1, :], 0.0)
                    nc.vector.memset(c3[:, s_c3(256):s_c3(256) + 1, :], 0.0)

                # ---- t0 (= Wav y + bav), full-image persistent, 4-row steps
                t0_hi = min(r1 + 4, 256)
                for g in range(t0_done, t0_hi, 2):
                    p2 = ps2.tile([D, 512], F32, tag="p2")
                    nc.tensor.matmul(p2, wavt_t,
                                     oy[0:D, oslot(g):oslot(g) + 2, :],
                                     start=True, stop=True, skip_group_check=True)
                    wr(t0[:, s_t0(g):s_t0(g) + 2, 1:257], p2, bias['bt0'], False)
                t0_done = t0_hi

                # ---- software-pipelined conv waves: interleave the four
                # convs with a 2-wave lag so PE never waits on stage writes
                def do_conv(src, s_src, wpre, g, sz, stop_last):
                    ssz = sz // 2
                    p = ps.tile([2 * D, 128 * sz], F32, tag="cv")
                    for i, sig in enumerate((g - 1, g + 1)):
                        sl = s_src(sig)
                        for kx in range(3):
                            mv = src[:, sl:sl + 2 * ssz, kx:kx + 256]
                            mv = mv.rearrange("p (s t) c -> p t s c", t=2)
                            nc.tensor.matmul(p, wdr[f"{wpre}{kx}{'ab'[i]}"], mv,
                                             start=(i == 0 and kx == 0),
                                             stop=(stop_last and i == 1 and kx == 2),
                                             perf_mode=DR, skip_group_check=True)
                    return p

                def do1(g, sz):
                    ssz = sz // 2
                    p = do_conv(t0, s_t0, "w1", g, sz, True)
                    for rho in range(2):
                        dst = c1[:, s_c1(g + rho):s_c1(g + rho) + 2 * ssz:2, 1:257]
                        wr(dst, p[rho * D:(rho + 1) * D, :], bias['bc1'], True)

                def do2(g, sz):
                    ssz = sz // 2
                    p = do_conv(c1, s_c1, "w2", g, sz, False)
                    for rho in range(2):
                        nc.tensor.matmul(
                            p[rho * D:(rho + 1) * D, :], wavt_t,
                            oy[0:D, oslot(g + rho):oslot(g + rho) + 2 * ssz:2, :],
                            start=False, stop=(rho == 1), skip_group_check=True)
                    for rho in range(2):
                        psl = p[rho * D:(rho + 1) * D, :]
                        dstb = oy[D:2 * D, oslot(g + rho):oslot(g + rho) + 2 * ssz:2, :]
                        wr(dstb, psl, bias['bo3b'], False)
                        # fp8 copy for conv3 input: GPSIMD from the bf16 o3
                        # (o3_bf16 carries +b4; subtract it again here)
                        dst = o3f[:, s_o3(g + rho):s_o3(g + rho) + 2 * ssz:2, 1:257]
                        nc.gpsimd.tensor_scalar_add(out=dst, in0=dstb,
                                                    scalar1=bias['bn4'])

                def do3(g, sz):
                    ssz = sz // 2
                    p = do_conv(o3f, s_o3, "w3", g, sz, True)
                    for rho in range(2):
                        dst = c3[:, s_c3(g + rho):s_c3(g + rho) + 2 * ssz:2, 1:257]
                        wr(dst, p[rho * D:(rho + 1) * D, :], bias['bc3'], True)

                ot_box = [None]

                def do4(j):
                    g = r0 + 4 * j
                    if j % 4 == 0:
                        ot_box[0] = outs.tile([2 * D, 8, W_IMG], F32, tag="ot", name="ot")
                    ot = ot_box[0]
                    p = do_conv(c3, s_c3, "w4", g, 4, False)
                    for rho in range(2):
                        nc.tensor.matmul(
                            p[rho * D:(rho + 1) * D, :], ii_t,
                            oy[:, oslot(g + rho):oslot(g + rho) + 4:2, :],
                            start=False, stop=(rho == 1), skip_group_check=True)
                    s0 = (g - (r0 + 16 * (j // 4))) // 2
                    dst = ot[:, s0:s0 + 2, :]
                    n = dst.free_size()
                    ca, cd = n * 0.833 + 143.0, n * 1.042 + 125.0
                    if acc[0] + ca <= acc[1] + cd:
                        acc[0] += ca
                        nc.scalar.activation(out=dst, in_=p, func=AF.Copy,
                                             bias=0.0, scale=1.0)
                    else:
                        acc[1] += cd
                        nc.vector.tensor_copy(out=dst, in_=p)
                    if j % 4 == 3:
                        gq = r0 + 16 * (j // 4)
                        nc.sync.dma_start(out=out_d[:, gq:gq + 15:2, :],
                                          in_=ot[0:D, :, :])
                        nc.sync.dma_start(out=out_d[:, gq + 1:gq + 16:2, :],
                                          in_=ot[D:2 * D, :, :])

                L1 = _emit_groups(c1lo, c1hi)
                L2 = _emit_groups(o3lo, o3hi)
                L3 = _emit_groups(c3lo, c3hi)
                n4 = R // 4
                for w in range(len(L1) + 7):
                    if w < len(L1):
                        do1(*L1[w])
                    if 0 <= w - 2 < len(L2):
                        do2(*L2[w - 2])
                    if 0 <= w - 4 < len(L3):
                        do3(*L3[w - 4])
                    if 0 <= w - 6 < n4:
                        do4(w - 6)
    nc.compile()
    return nc


_NC_CACHE = {}


def _get_ncs():
    if "a" not in _NC_CACHE:
        _NC_CACHE["a"] = _build_nc_a()
        _NC_CACHE["b"] = _build_nc_b()
    return _NC_CACHE["a"], _NC_CACHE["b"]


def _host_fold(G, Sx, Sy, Wq, bq, Wk, bk, Vw, vb):
    """Raw Gram [128,128] + channel sums -> (Wav [64,64], bav [64]) in f64."""
    G = G.astype(np.float64)
    Gxx, Gxy, Gyy = G[:D, :D], G[:D, D:], G[D:, D:]
    n = float(HW)
    QK = (Wq @ Gxy @ Wk.T + np.outer(Wq @ Sx, bk)
          + np.outer(bq, Wk @ Sy) + n * np.outer(bq, bk))
    qq = np.einsum('ij,jk,ik->i', Wq, Gxx, Wq) + 2 * bq * (Wq @ Sx) + n * bq * bq
    kk = np.einsum('ij,jk,ik->i', Wk, Gyy, Wk) + 2 * bk * (Wk @ Sy) + n * bk * bk
    St = QK / np.maximum(np.sqrt(qq), 1e-12)[:, None] \
            / np.maximum(np.sqrt(kk), 1e-12)[None, :]
    A = np.zeros((D, D))
    for h in range(4):
        blk = St[16 * h:16 * h + 16, 16 * h:16 * h + 16]
        e = np.exp(blk - blk.max(axis=1, keepdims=True))
        A[16 * h:16 * h + 16, 16 * h:16 * h + 16] = e / e.sum(axis=1, keepdims=True)
    return A @ Vw, A @ vb


def _prep_dr_weights(w):
    """w [64o, 64i, 3, 3] f32 -> dict kx -> (Wa, Wb) [64, 2, 128] e4m3."""
    out = {}
    for kx in range(3):
        Wa = np.zeros((D, 2, 2 * D), np.float32)
        Wb = np.zeros((D, 2, 2 * D), np.float32)
        wt = w[:, :, :, kx]  # [o, i, ky]
        Wa[:, 0, 0:D] = wt[:, :, 0].T
        Wa[:, 1, 0:D] = wt[:, :, 1].T
        Wa[:, 1, D:2 * D] = wt[:, :, 0].T
        Wb[:, 0, 0:D] = wt[:, :, 2].T
        Wb[:, 0, D:2 * D] = wt[:, :, 1].T
        Wb[:, 1, D:2 * D] = wt[:, :, 2].T
        out[kx] = (Wa.astype(E4), Wb.astype(E4))
    return out


def kernel(x, y, qw, qb, kw, kb, vw, vb,
           r1w1, r1b1, r1w2, r1b2, r2w1, r2b1, r2w2, r2b2, **_):
    x = np.asarray(x, np.float32)
    y = np.asarray(y, np.float32)
    qw, qb, kw, kb = (np.asarray(a, np.float32) for a in (qw, qb, kw, kb))
    vw, vb = np.asarray(vw, np.float32), np.asarray(vb, np.float32)
    r1w1, r1b1, r1w2, r1b2 = (np.asarray(a, np.float32) for a in (r1w1, r1b1, r1w2, r1b2))
    r2w1, r2b1, r2w2, r2b2 = (np.asarray(a, np.float32) for a in (r2w1, r2b1, r2w2, r2b2))
    nca, ncb = _get_ncs()

    # ---- pass A: pixel-major Gram
    in_maps_a = []
    xs_l, ys_l = [], []
    for c in range(NCORES):
        xc = x[c].reshape(D, HW)
        yc = y[c].reshape(D, HW)
        xs_l.append(xc.sum(axis=1, dtype=np.float64))
        ys_l.append(yc.sum(axis=1, dtype=np.float64))
        Z = np.empty((HW, 2 * D), np.float32)
        Z[:, :D] = xc.T
        Z[:, D:] = yc.T
        za = Z.reshape(32, 16, 128, 128).transpose(0, 2, 1, 3) \
              .reshape(32, 128, 2048).astype(BF)
        in_maps_a.append({"za": np.ascontiguousarray(za)})
    res_a = run_bass_kernel_spmd(nca, in_maps_a, core_ids=list(range(NCORES)))

    # ---- host fold + pass-B constants
    Wq, Wk, Vw = qw[:, :, 0, 0].astype(np.float64), kw[:, :, 0, 0].astype(np.float64), \
        vw[:, :, 0, 0].astype(np.float64)
    bq64, bk64, vb64 = qb.astype(np.float64), kb.astype(np.float64), vb.astype(np.float64)
    wdr_np = {}
    for ci, w in ((1, r1w1), (2, r1w2), (3, r2w1), (4, r2w2)):
        d = _prep_dr_weights(w)
        for kx in range(3):
            wdr_np[f"w{ci}{kx}a"] = d[kx][0]
            wdr_np[f"w{ci}{kx}b"] = d[kx][1]
    ii = np.concatenate([np.eye(D, dtype=np.float32)] * 2, axis=0).astype(BF)

    in_maps_b = []
    for c in range(NCORES):
        Wav, bav = _host_fold(res_a.results[c]["gout"], xs_l[c], ys_l[c],
                              Wq, bq64, Wk, bk64, Vw, vb64)
        m = {"yb": np.ascontiguousarray(y[c].reshape(D, H, W_IMG).astype(BF)),
             "wavt": np.ascontiguousarray(Wav.T.astype(np.float32).astype(BF)),
             "ii": ii,
             "bt0": bav.astype(np.float32).reshape(D, 1),
             "bc1": r1b1.reshape(D, 1),
             "bo3b": (bav + r1b2 + r2b2).astype(np.float32).reshape(D, 1),
             "bn4": (-r2b2).astype(np.float32).reshape(D, 1),
             "bc3": r2b1.reshape(D, 1)}
        m.update(wdr_np)
        in_maps_b.append({k: np.ascontiguousarray(v) for k, v in m.items()})
    res_b = run_bass_kernel_spmd(ncb, in_maps_b, core_ids=list(range(NCORES)))

    return np.stack([res_b.results[c]["out"].reshape(D, H, W_IMG)
                     for c in range(NCORES)]).astype(np.float32)


if __name__ == "__main__":
    rng = np.random.default_rng(0)
    ins = {
        "x": rng.standard_normal((8, D, H, W_IMG)).astype(np.float32),
        "y": rng.standard_normal((8, D, H, W_IMG)).astype(np.float32),
        "qw": (rng.standard_normal((D, D, 1, 1)) / 8).astype(np.float32),
        "qb": (rng.standard_normal(D) / 8).astype(np.float32),
        "kw": (rng.standard_normal((D, D, 1, 1)) / 8).astype(np.float32),
        "kb": (rng.standard_normal(D) / 8).astype(np.float32),
        "vw": (rng.standard_normal((D, D, 1, 1)) / 8).astype(np.float32),
        "vb": (rng.standard_normal(D) / 8).astype(np.float32),
    }
    for i in (1, 2):
        for j in (1, 2):
            ins[f"r{i}w{j}"] = (rng.standard_normal((D, D, 3, 3)) / 24).astype(np.float32)
            ins[f"r{i}b{j}"] = (rng.standard_normal(D) / 24).astype(np.float32)
    o = kernel(**ins)
    print("kernel ran, out shape", o.shape, "std", o.std())


# revision 35
# speedup vs baseline: 1.0058x; 1.0058x over previous
"""Trainium2 Bass kernel for channel-attention + 2 residual conv blocks.

Data-parallel over batch (8 cores, 1 batch element each). Two SPMD launches:
  A) raw channel Gram G = [x;y]^T-pixel-contraction ([128,128]) via 512
     accumulating matmuls on pixel-major data (host pre-transposes); no
     per-tile PSUM->SBUF copies at all.
  B) fused attention-apply + 4 3x3 convs. Convs run in fp8-e4m3 DoubleRow
     matmuls (2 k-tiles replace the row-shifted duplicate half entirely),
     packed 4 output rows per matmul group (M=128=2rows x 64ch,
     N=512=2 slot-pairs x 256 cols). Residual paths stay exact via bf16
     identity/Wav injection matmuls into PSUM; stage writes balance across
     ACT/DVE (GPSIMD cannot read PSUM) and the four convs are
     software-pipelined in 2-wave-lagged groups per 32-row block. Out rows
     stage parity-paired in SBUF and leave in 16-row DMAs. Host does only
     the O(64^2) softmax/fold algebra between launches.
"""
import sys, os
for p in ('/opt/trn_rl_repo', os.path.expanduser('~/.axon_site/_ro/trn_rl_repo')):
    if os.path.isdir(p) and p not in sys.path:
        sys.path.insert(0, p)

import numpy as np
import ml_dtypes
import concourse.bass as bass
import concourse.bacc as bacc
import concourse.tile as tile
from concourse import mybir
from concourse.bass_utils import run_bass_kernel_spmd

dt = mybir.dt
F32, BF16, FP8 = dt.float32, dt.bfloat16, dt.float8e4
BF = ml_dtypes.bfloat16
E4 = ml_dtypes.float8_e4m3
AF = mybir.ActivationFunctionType
OP = mybir.AluOpType
DR = mybir.MatmulPerfMode.DoubleRow

D = 64
HW = 65536
H = W_IMG = 256
NCORES = 8
R = 32  # output rows per block in pass B


def _build_nc_a():
    nc = bacc.Bacc("TRN2", target_bir_lowering=False, debug=False)
    za = nc.dram_tensor("za", [32, 128, 2048], BF16, kind="ExternalInput").ap()
    gout = nc.dram_tensor("gout", [128, 128], F32, kind="ExternalOutput").ap()
    with tile.TileContext(nc) as tc:
        with tc.tile_pool(name="io", bufs=3) as io, \
             tc.tile_pool(name="work", bufs=1) as work, \
             tc.tile_pool(name="gps", bufs=1, space="PSUM") as gps:
            gp = gps.tile([128, 128], F32)
            for t in range(32):
                zt = io.tile([128, 2048], BF16, tag="zt")
                nc.sync.dma_start(out=zt, in_=za[t])
                for j in range(16):
                    s = zt[:, j * 128:(j + 1) * 128]
                    nc.tensor.matmul(gp, s, s,
                                     start=(t == 0 and j == 0),
                                     stop=(t == 31 and j == 15),
                                     skip_group_check=True)
            gs = work.tile([128, 128], F32)
            nc.vector.tensor_copy(out=gs, in_=gp)
            nc.sync.dma_start(out=gout, in_=gs)
    nc.compile()
    return nc


def _emit_groups(lo, hi):
    """4-row groups (+2-row remainder; odd counts overlap by one row)."""
    out, g, n = [], lo, hi - lo
    while n >= 4:
        out.append((g, 4)); g += 4; n -= 4
    if n == 3:
        out.append((hi - 4, 4))
    elif n == 2:
        out.append((g, 2))
    elif n == 1:
        out.append((hi - 2, 2))
    return out


def _build_nc_b():
    nc = bacc.Bacc("TRN2", target_bir_lowering=False, debug=False)
    yb = nc.dram_tensor("yb", [D, H, W_IMG], BF16, kind="ExternalInput").ap()
    wavt = nc.dram_tensor("wavt", [D, D], BF16, kind="ExternalInput").ap()
    ii_d = nc.dram_tensor("ii", [2 * D, D], BF16, kind="ExternalInput").ap()
    # fp8 DoubleRow weights: per conv, per kx, (a|b) variant [64, 2, 128]
    wdr_d = {}
    for c in range(1, 5):
        for kx in range(3):
            for v in 'ab':
                nm = f"w{c}{kx}{v}"
                wdr_d[nm] = nc.dram_tensor(nm, [D, 2, 2 * D], FP8,
                                           kind="ExternalInput").ap()
    bias_d = {nm: nc.dram_tensor(nm, [D, 1], F32, kind="ExternalInput").ap()
              for nm in ('bt0', 'bc1', 'bo3b', 'bn4', 'bc3')}
    out_d = nc.dram_tensor("out", [D, H, W_IMG], F32, kind="ExternalOutput").ap()

    with tile.TileContext(nc) as tc:
        with tc.tile_pool(name="consts", bufs=1) as consts, \
             tc.tile_pool(name="stg", bufs=1) as stg, \
             tc.tile_pool(name="stgr", bufs=2) as stgr, \
             tc.tile_pool(name="oyp", bufs=2) as oyp, \
             tc.tile_pool(name="outs", bufs=2) as outs, \
             tc.tile_pool(name="ps", bufs=6, space="PSUM") as ps, \
             tc.tile_pool(name="ps2", bufs=2, space="PSUM") as ps2:
            # block-0 y load first: everything on the critical path needs it,
            # and the ~30 const DMAs would otherwise delay it on the SP queue
            oy0 = oyp.tile([2 * D, 40, W_IMG], BF16, tag="oy")
            nc.sync.dma_start(out=oy0[0:D, 4:16, :], in_=yb[:, 0:12, :])
            nc.sync.dma_start(out=oy0[0:D, 16:40, :], in_=yb[:, 12:36, :])

            # const loads issue from the (idle) Pool/ACT DMA queues so they
            # don't serialize behind each other or the y loads on SP
            wavt_t = consts.tile([D, D], BF16)
            ii_t = consts.tile([2 * D, D], BF16)
            nc.scalar.dma_start(out=wavt_t, in_=wavt)
            nc.scalar.dma_start(out=ii_t, in_=ii_d)
            wdr = {}
            for nm, d in wdr_d.items():
                t = consts.tile([D, 2, 2 * D], FP8, tag=nm)
                nc.gpsimd.dma_start(out=t, in_=d)
                wdr[nm] = t
            bias = {}
            for nm, d in bias_d.items():
                t = consts.tile([D, 1], F32, tag=nm)
                nc.scalar.dma_start(out=t, in_=d)
                bias[nm] = t

            # t0 persists across blocks (full image); col 0/257 stay zero
            t0 = stg.tile([D, 258, 258], FP8)   # slot = row + 1
            nc.vector.memset(t0[:, :, 0:1], 0.0)
            nc.vector.memset(t0[:, :, 257:258], 0.0)
            nc.vector.memset(t0[:, 0:1, :], 0.0)     # virtual row -1
            nc.vector.memset(t0[:, 257:258, :], 0.0)  # virtual row 256

            # stage-write engine balance: psum -> stage with bias (+relu).
            # GPSIMD cannot read PSUM, so these go to ACT/DVE, weighted by
            # modeled per-op cost (ACT 0.833ns/el +143, DVE 1.042ns/el +125).
            acc = [0.0, 0.0]

            def wr(out_ap, in_ap, b, relu):
                n = out_ap.free_size()
                ca, cd = n * 0.833 + 143.0, n * 1.042 + 125.0
                if acc[0] + ca <= acc[1] + cd:
                    acc[0] += ca
                    nc.scalar.activation(out=out_ap, in_=in_ap,
                                         func=(AF.Relu if relu else AF.Identity),
                                         bias=b, scale=1.0)
                elif relu:
                    acc[1] += cd
                    nc.vector.tensor_scalar(out=out_ap, in0=in_ap, scalar1=b,
                                            scalar2=0.0, op0=OP.add, op1=OP.max)
                else:
                    acc[1] += cd
                    nc.vector.tensor_scalar_add(out=out_ap, in0=in_ap, scalar1=b)

            # --- block loop -------------------------------------------------
            t0_done = 0  # t0 rows produced so far
            for blk in range(8):
                r0, r1 = blk * R, (blk + 1) * R
                oy = oy0 if blk == 0 else oyp.tile([2 * D, 40, W_IMG], BF16,
                                                   tag="oy")
                # per-block stages, double-buffered for cross-block overlap
                c1 = stgr.tile([D, 40, 258], FP8, tag="c1")
                o3f = stgr.tile([D, 38, 258], FP8, tag="o3f")
                c3 = stgr.tile([D, 36, 258], FP8, tag="c3")
                if blk < 2:  # each rotating buffer's gap cols, zeroed once
                    for t in (c1, o3f, c3):
                        nc.vector.memset(t[:, :, 0:1], 0.0)
                        nc.vector.memset(t[:, :, 257:258], 0.0)

                def oslot(row):
                    return row - (r0 - 4)

                if blk > 0:
                    ylo, yhi = r0 - 4, min(r1 + 4, 256)
                    nc.sync.dma_start(out=oy[0:D, oslot(ylo):oslot(yhi), :],
                                      in_=yb[:, ylo:yhi, :])

                # stage ranges (produced rows)
                c1lo, c1hi = max(r0 - 3, 0), min(r1 + 3, 256)
                o3lo, o3hi = max(r0 - 2, 0), min(r1 + 2, 256)
                c3lo, c3hi = max(r0 - 1, 0), min(r1 + 1, 256)

                def s_c1(row): return row - c1lo + 1
                def s_o3(row): return row - o3lo + 1
                def s_c3(row): return row - c3lo + 1
                def s_t0(row): return row + 1

                # virtual zero rows at image edges (persistent tiles: emit
                # only when the slot is actually consumed as a virtual row)
                if blk == 0:
                    for t in (c1, o3f, c3):
                        nc.vector.memset(t[:, 0:1, :], 0.0)
                if blk == 7:
                    nc.vector.memset(c1[:, s_c1(256):s_c1(256) + 1, :], 0.0)
                    nc.vector.memset(o3f[:, s_o3(256):s_o3(256) + 1, :], 0.0)
                    nc.vector.memset(c3[:, s_c3(256):s_c3(256) + 1, :], 0.0)

                # ---- t0 (= Wav y + bav), full-image persistent, 4-row steps
                t0_hi = min(r1 + 4, 256)
                for g in range(t0_done, t0_hi, 2):
                    p2 = ps2.tile([D, 512], F32, tag="p2")
                    nc.tensor.matmul(p2, wavt_t,
                                     oy[0:D, oslot(g):oslot(g) + 2, :],
                                     start=True, stop=True, skip_group_check=True)
                    wr(t0[:, s_t0(g):s_t0(g) + 2, 1:257], p2, bias['bt0'], False)
                t0_done = t0_hi

                # ---- software-pipelined conv waves: interleave the four
                # convs with a 2-wave lag so PE never waits on stage writes
                def do_conv(src, s_src, wpre, g, sz, stop_last):
                    ssz = sz // 2
                    p = ps.tile([2 * D, 128 * sz], F32, tag="cv")
                    for i, sig in enumerate((g - 1, g + 1)):
                        sl = s_src(sig)
                        for kx in range(3):
                            mv = src[:, sl:sl + 2 * ssz, kx:kx + 256]
                            mv = mv.rearrange("p (s t) c -> p t s c", t=2)
                            nc.tensor.matmul(p, wdr[f"{wpre}{kx}{'ab'[i]}"], mv,
                                             start=(i == 0 and kx == 0),
                                             stop=(stop_last and i == 1 and kx == 2),
                                             perf_mode=DR, skip_group_check=True)
                    return p

                def do1(g, sz):
                    ssz = sz // 2
                    p = do_conv(t0, s_t0, "w1", g, sz, True)
                    for rho in range(2):
                        dst = c1[:, s_c1(g + rho):s_c1(g + rho) + 2 * ssz:2, 1:257]
                        wr(dst, p[rho * D:(rho + 1) * D, :], bias['bc1'], True)

                def do2(g, sz):
                    ssz = sz // 2
                    p = do_conv(c1, s_c1, "w2", g, sz, False)
                    for rho in range(2):
                        nc.tensor.matmul(
                            p[rho * D:(rho + 1) * D, :], wavt_t,
                            oy[0:D, oslot(g + rho):oslot(g + rho) + 2 * ssz:2, :],
                            start=False, stop=(rho == 1), skip_group_check=True)
                    for rho in range(2):
                        psl = p[rho * D:(rho + 1) * D, :]
                        dstb = oy[D:2 * D, oslot(g + rho):oslot(g + rho) + 2 * ssz:2, :]
                        wr(dstb, psl, bias['bo3b'], False)
                        # fp8 copy for conv3 input: GPSIMD from the bf16 o3
                        # (o3_bf16 carries +b4; subtract it again here)
                        dst = o3f[:, s_o3(g + rho):s_o3(g + rho) + 2 * ssz:2, 1:257]
                        nc.gpsimd.tensor_scalar_add(out=dst, in0=dstb,
                                                    scalar1=bias['bn4'])

                def do3(g, sz):
                    ssz = sz // 2
                    p = do_conv(o3f, s_o3, "w3", g, sz, True)
                    for rho in range(2):
                        dst = c3[:, s_c3(g + rho):s_c3(g + rho) + 2 * ssz:2, 1:257]
                        wr(dst, p[rho * D:(rho + 1) * D, :], bias['bc3'], True)

                ot_box = [None]

                def do4(j):
                    g = r0 + 4 * j
                    if j % 2 == 0:
                        ot_box[0] = outs.tile([2 * D, 4, W_IMG], F32, tag="ot", name="ot")
                    ot = ot_box[0]
                    p = do_conv(c3, s_c3, "w4", g, 4, False)
                    for rho in range(2):
                        nc.tensor.matmul(
                            p[rho * D:(rho + 1) * D, :], ii_t,
                            oy[:, oslot(g + rho):oslot(g + rho) + 4:2, :],
                            start=False, stop=(rho == 1), skip_group_check=True)
                    s0 = (g - (r0 + 8 * (j // 2))) // 2
                    dst = ot[:, s0:s0 + 2, :]
                    n = dst.free_size()
                    ca, cd = n * 0.833 + 143.0, n * 1.042 + 125.0
                    if acc[0] + ca <= acc[1] + cd:
                        acc[0] += ca
                        nc.scalar.activation(out=dst, in_=p, func=AF.Copy,
                                             bias=0.0, scale=1.0)
                    else:
                        acc[1] += cd
                        nc.vector.tensor_copy(out=dst, in_=p)
                    if j % 2 == 1:
                        gq = r0 + 8 * (j // 2)
                        nc.sync.dma_start(out=out_d[:, gq:gq + 7:2, :],
                                          in_=ot[0:D, :, :])
                        nc.sync.dma_start(out=out_d[:, gq + 1:gq + 8:2, :],
                                          in_=ot[D:2 * D, :, :])

                L1 = _emit_groups(c1lo, c1hi)
                L2 = _emit_groups(o3lo, o3hi)
                L3 = _emit_groups(c3lo, c3hi)
                n4 = R // 4
                for w in range(len(L1) + 7):
                    if w < len(L1):
                        do1(*L1[w])
                    if 0 <= w - 2 < len(L2):
                        do2(*L2[w - 2])
                    if 0 <= w - 4 < len(L3):
                        do3(*L3[w - 4])
                    if 0 <= w - 6 < n4:
                        do4(w - 6)
    nc.compile()
    return nc


_NC_CACHE = {}


def _get_ncs():
    if "a" not in _NC_CACHE:
        _NC_CACHE["a"] = _build_nc_a()
        _NC_CACHE["b"] = _build_nc_b()
    return _NC_CACHE["a"], _NC_CACHE["b"]


def _host_fold(G, Sx, Sy, Wq, bq, Wk, bk, Vw, vb):
    """Raw Gram [128,128] + channel sums -> (Wav [64,64], bav [64]) in f64."""
    G = G.astype(np.float64)
    Gxx, Gxy, Gyy = G[:D, :D], G[:D, D:], G[D:, D:]
    n = float(HW)
    QK = (Wq @ Gxy @ Wk.T + np.outer(Wq @ Sx, bk)
          + np.outer(bq, Wk @ Sy) + n * np.outer(bq, bk))
    qq = np.einsum('ij,jk,ik->i', Wq, Gxx, Wq) + 2 * bq * (Wq @ Sx) + n * bq * bq
    kk = np.einsum('ij,jk,ik->i', Wk, Gyy, Wk) + 2 * bk * (Wk @ Sy) + n * bk * bk
    St = QK / np.maximum(np.sqrt(qq), 1e-12)[:, None] \
            / np.maximum(np.sqrt(kk), 1e-12)[None, :]
    A = np.zeros((D, D))
    for h in range(4):
        blk = St[16 * h:16 * h + 16, 16 * h:16 * h + 16]
        e = np.exp(blk - blk.max(axis=1, keepdims=True))
        A[16 * h:16 * h + 16, 16 * h:16 * h + 16] = e / e.sum(axis=1, keepdims=True)
    return A @ Vw, A @ vb


def _prep_dr_weights(w):
    """w [64o, 64i, 3, 3] f32 -> dict kx -> (Wa, Wb) [64, 2, 128] e4m3."""
    out = {}
    for kx in range(3):
        Wa = np.zeros((D, 2, 2 * D), np.float32)
        Wb = np.zeros((D, 2, 2 * D), np.float32)
        wt = w[:, :, :, kx]  # [o, i, ky]
        Wa[:, 0, 0:D] = wt[:, :, 0].T
        Wa[:, 1, 0:D] = wt[:, :, 1].T
        Wa[:, 1, D:2 * D] = wt[:, :, 0].T
        Wb[:, 0, 0:D] = wt[:, :, 2].T
        Wb[:, 0, D:2 * D] = wt[:, :, 1].T
        Wb[:, 1, D:2 * D] = wt[:, :, 2].T
        out[kx] = (Wa.astype(E4), Wb.astype(E4))
    return out


def kernel(x, y, qw, qb, kw, kb, vw, vb,
           r1w1, r1b1, r1w2, r1b2, r2w1, r2b1, r2w2, r2b2, **_):
    x = np.asarray(x, np.float32)
    y = np.asarray(y, np.float32)
    qw, qb, kw, kb = (np.asarray(a, np.float32) for a in (qw, qb, kw, kb))
    vw, vb = np.asarray(vw, np.float32), np.asarray(vb, np.float32)
    r1w1, r1b1, r1w2, r1b2 = (np.asarray(a, np.float32) for a in (r1w1, r1b1, r1w2, r1b2))
    r2w1, r2b1, r2w2, r2b2 = (np.asarray(a, np.float32) for a in (r2w1, r2b1, r2w2, r2b2))
    nca, ncb = _get_ncs()

    # ---- pass A: pixel-major Gram
    in_maps_a = []
    xs_l, ys_l = [], []
    for c in range(NCORES):
        xc = x[c].reshape(D, HW)
        yc = y[c].reshape(D, HW)
        xs_l.append(xc.sum(axis=1, dtype=np.float64))
        ys_l.append(yc.sum(axis=1, dtype=np.float64))
        Z = np.empty((HW, 2 * D), np.float32)
        Z[:, :D] = xc.T
        Z[:, D:] = yc.T
        za = Z.reshape(32, 16, 128, 128).transpose(0, 2, 1, 3) \
              .reshape(32, 128, 2048).astype(BF)
        in_maps_a.append({"za": np.ascontiguousarray(za)})
    res_a = run_bass_kernel_spmd(nca, in_maps_a, core_ids=list(range(NCORES)))

    # ---- host fold + pass-B constants
    Wq, Wk, Vw = qw[:, :, 0, 0].astype(np.float64), kw[:, :, 0, 0].astype(np.float64), \
        vw[:, :, 0, 0].astype(np.float64)
    bq64, bk64, vb64 = qb.astype(np.float64), kb.astype(np.float64), vb.astype(np.float64)
    wdr_np = {}
    for ci, w in ((1, r1w1), (2, r1w2), (3, r2w1), (4, r2w2)):
        d = _prep_dr_weights(w)
        for kx in range(3):
            wdr_np[f"w{ci}{kx}a"] = d[kx][0]
            wdr_np[f"w{ci}{kx}b"] = d[kx][1]
    ii = np.concatenate([np.eye(D, dtype=np.float32)] * 2, axis=0).astype(BF)

    in_maps_b = []
    for c in range(NCORES):
        Wav, bav = _host_fold(res_a.results[c]["gout"], xs_l[c], ys_l[c],
                              Wq, bq64, Wk, bk64, Vw, vb64)
        m = {"yb": np.ascontiguousarray(y[c].reshape(D, H, W_IMG).astype(BF)),
             "wavt": np.ascontiguousarray(Wav.T.astype(np.float32).astype(BF)),
             "ii": ii,
             "bt0": bav.astype(np.float32).reshape(D, 1),
             "bc1": r1b1.reshape(D, 1),
             "bo3b": (bav + r1b2 + r2b2).astype(np.float32).reshape(D, 1),
             "bn4": (-r2b2).astype(np.float32).reshape(D, 1),
             "bc3": r2b1.reshape(D, 1)}
        m.update(wdr_np)
        in_maps_b.append({k: np.ascontiguousarray(v) for k, v in m.items()})
    res_b = run_bass_kernel_spmd(ncb, in_maps_b, core_ids=list(range(NCORES)))

    return np.stack([res_b.results[c]["out"].reshape(D, H, W_IMG)
                     for c in range(NCORES)]).astype(np.float32)


if __name__ == "__main__":
    rng = np.random.default_rng(0)
    ins = {
        "x": rng.standard_normal((8, D, H, W_IMG)).astype(np.float32),
        "y": rng.standard_normal((8, D, H, W_IMG)).astype(np.float32),
        "qw": (rng.standard_normal((D, D, 1, 1)) / 8).astype(np.float32),
        "qb": (rng.standard_normal(D) / 8).astype(np.float32),
        "kw": (rng.standard_normal((D, D, 1, 1)) / 8).astype(np.float32),
        "kb": (rng.standard_normal(D) / 8).astype(np.float32),
        "vw": (rng.standard_normal((D, D, 1, 1)) / 8).astype(np.float32),
        "vb": (rng.standard_normal(D) / 8).astype(np.float32),
    }
    for i in (1, 2):
        for j in (1, 2):
            ins[f"r{i}w{j}"] = (rng.standard_normal((D, D, 3, 3)) / 24).astype(np.float32)
            ins[f"r{i}b{j}"] = (rng.standard_normal(D) / 24).astype(np.float32)
    o = kernel(**ins)
    print("kernel ran, out shape", o.shape, "std", o.std())
